# revision 39
# baseline (speedup 1.0000x reference)
"""Bundle-adjustment forward projection on 8 Trainium2 NeuronCores.

reference:  R = euler_to_matrix(euler_angles)            [V,3,3]
            pc = einsum('nj,vij->vni', points3d, R) + t  [V,N,3]
            Zc = min(pc_z, -1e-4)
            u = -f*Xc/Zc + CX ; v = f*Yc/Zc + CY         -> [V,N,2]

Strategy: shard the N=200000 points across the 8 cores (25000 each); every
core computes all V=128 views so the SBUF partition dim = view is fully
utilized.  The host folds f/CX/CY/translations into three linear maps over
homogeneous points p4 = (x,y,z,1):

    U     = p4 . Wu[v]   ( =  f*Xc + CX*znega )
    Vv    = p4 . Wv[v]   ( = -f*Yc + CY*znega )
    znega = p4 . Wz[v]   ( = -Zc, always > 0 for this data )

so that u = U/znega and v = Vv/znega exactly match the reference when the
Z clamp never fires (host-verified with a rigorous bound; a clamped variant
is built instead if the bound is violated).

Per 500-point chunk: 3 PE matmuls (float32r, K=4, M=128 views, weights
resident in three PE row groups) -> PSUM; DVE: reciprocal_approx_fast on
znega, then two tensor_muls writing u,v interleaved (stride-2) straight into
the output tile; one 2.56 MB DMA per 2500 points stores [128, 5000] f32.

NOTE this walrus build accepts only ONE semaphore wait per instruction, so
the kernel is structured so every instruction has at most one unobserved
producer (points+weights share one input DMA per row group; the per-chunk
DVE stream leads with the reciprocal so PE ticks are observed before the
muls; TileContext's tail drain is patched to split its waits into nops).
"""

import numpy as np
from contextlib import ExitStack

import concourse.bass as bass
import concourse.tile as tile
from concourse import mybir
from concourse.bass_utils import run_bass_kernel_spmd
from concourse.vector_clock import ScopedClock, VectorClock

CX = 512.0
CY = 512.0
Z_MAX = -1e-4

N_CORES = 8
N_POINTS = 200000
N_VIEWS = 128
NPC = N_POINTS // N_CORES          # 25000 points per core
CHUNK = 500                        # matmul free dim (fits one PSUM bank, >=256)
CHUNKS = NPC // CHUNK              # 50
# outputs are stored per chunk (512 KB each) so the DMA queues always have
# work; gtile groups exist only for SBUF slot management
GSCHED = [5] * 10
assert sum(GSCHED) == CHUNKS
GMAX = max(GSCHED)
# blob0 carries TWO weight sections (u then v) followed by the shared point
# columns; blob_z carries one.  11-partition loads land on only 3 of the 16
# SBUF ports (~81 GB/s), so sharing one rhs replica between u and v cuts the
# input from 1.66 MB to 1.11 MB of port-bound traffic.
W0 = 2 * N_VIEWS
BLOB0 = W0 + NPC
BLOBZ = N_VIEWS + NPC
BLOB = NPC + N_VIEWS               # points ++ weight columns
# bf16 hi/lo split: K rows = [p_hi(3), p_lo(3), p_hi(3), 1, 1] against
# weight columns [w_hi(3), w_hi(3), w_lo(3), b_hi, b_lo].  All products are
# exact in the fp32 PSUM accumulate; only w_lo*p_lo (~2^-18 relative) is
# dropped -- ~30x more accurate than float32r and full PE rate.
KROWS = 11

F32 = mybir.dt.float32
BF16 = mybir.dt.bfloat16


# ---------------------------------------------------------------------------
# Tile tail-drain workaround: this walrus build only accepts ONE semaphore
# wait per CTRL instruction, but TileContext puts every outstanding proc's
# wait on the single tail Drain.  Emit one-wait nops first instead.
# ---------------------------------------------------------------------------
def _split_drain_and_barrier(self, tick_clock, wait_clock):
    gc = tick_clock.global_clock
    n = len(gc)
    for p in range(n):
        if gc[p] > 0:
            vec = [0] * n
            vec[p] = gc[p]
            nop = self.nc.sync.nop()
            wait_clock.add_sem_waits(nop.ins, ScopedClock({None: VectorClock(vec)}))
    self.nc.sync.drain()
    self.nc.all_engine_barrier()
    assert self.sems is not None
    popped = self.nc._tile_sem_poison_stack.pop()
    assert popped is self._sem_poison
    self.nc.clear_and_free_semaphores(list(self.sems.allocated().values()))
    self.nc.all_engine_barrier()


tile.TileContext._drain_and_barrier = _split_drain_and_barrier


def _legalize_waits(bir: bytes) -> bytes:
    """This walrus build accepts at most ONE semaphore wait per instruction.
    Split every multi-wait instruction by injecting same-engine NoOps (each
    carrying one wait) immediately before it: engines consume their block
    instructions in order, so the nop's wait completes before the real op."""
    import json as _json

    d = _json.loads(bir)
    ctr = 0
    for f in d["functions"]:
        for b in f["blocks"]:
            newl = []
            for inst in b["instructions"]:
                si = inst.get("sync_info")
                w = (si or {}).get("on_wait") or []
                if len(w) > 1:
                    for extra in w[:-1]:
                        ctr += 1
                        newl.append(
                            {
                                "debug": inst.get("debug", 0),
                                "engine": inst["engine"],
                                "ins": [],
                                "outs": [],
                                "name": f"I-wfix{ctr}",
                                "opcode": "NoOp",
                                "sync_info": {"on_update": [], "on_wait": [extra]},
                            }
                        )
                    si["on_wait"] = [w[-1]]
                newl.append(inst)
            b["instructions"] = newl
    return _json.dumps(d).encode()


def _install_wait_legalizer(nc):
    orig = nc.to_json_bytes

    def to_json_bytes_fixed():
        return _legalize_waits(orig())

    nc.to_json_bytes = to_json_bytes_fixed
    return nc


# ---------------------------------------------------------------------------
# Host-side math
# ---------------------------------------------------------------------------
def _euler_to_matrix(e):
    """[V,3] -> [V,3,3], Rx @ Ry @ Rz (same convention as the reference)."""
    x, y, z = e[:, 0], e[:, 1], e[:, 2]
    c1, s1 = np.cos(x), np.sin(x)
    c2, s2 = np.cos(y), np.sin(y)
    c3, s3 = np.cos(z), np.sin(z)
    zero = np.zeros_like(x)
    one = np.ones_like(x)
    Rx = np.stack([one, zero, zero, zero, c1, -s1, zero, s1, c1], -1).reshape(-1, 3, 3)
    Ry = np.stack([c2, zero, s2, zero, one, zero, -s2, zero, c2], -1).reshape(-1, 3, 3)
    Rz = np.stack([c3, -s3, zero, s3, c3, zero, zero, zero, one], -1).reshape(-1, 3, 3)
    return Rx @ Ry @ Rz


def _fold_weights(euler_angles, translations, focal_length, clamp):
    """Build the three [4, V] stationary matrices (rows x,y,z,1)."""
    R = _euler_to_matrix(euler_angles.astype(np.float64))
    t = translations.astype(np.float64)
    f = float(focal_length[0])
    r0, r1, r2 = R[:, 0, :], R[:, 1, :], R[:, 2, :]
    tx, ty, tz = t[:, 0], t[:, 1], t[:, 2]

    if clamp:
        # numerators without the CX/CY fold (added on DVE after the division)
        wU = f * r0
        bU = f * tx
        wV = -f * r1
        bV = -f * ty
    else:
        wU = f * r0 - CX * r2
        bU = f * tx - CX * tz
        wV = -f * r1 - CY * r2
        bV = -f * ty - CY * tz
    wZ = -r2
    bZ = -tz

    def pack(w, b):
        # -> [KROWS, V] bf16 lhsT: cols per view = [w_hi(3), w_hi(3), w_lo(3),
        # b_hi, b_lo] matching point rows [p_hi(3), p_lo(3), p_hi(3), 1, 1]
        import ml_dtypes

        w_hi = w.astype(ml_dtypes.bfloat16)
        w_lo = (w - w_hi.astype(np.float64)).astype(ml_dtypes.bfloat16)
        b_hi = b.astype(ml_dtypes.bfloat16)
        b_lo = (b - b_hi.astype(np.float64)).astype(ml_dtypes.bfloat16)
        return np.concatenate(
            [w_hi.T, w_hi.T, w_lo.T, b_hi[None, :], b_lo[None, :]], axis=0
        )

    return pack(wU, bU), pack(wV, bV), pack(wZ, bZ)


# ---------------------------------------------------------------------------
# Bass module
# ---------------------------------------------------------------------------
def _build_module(clamp):
    nc = bass.Bass()
    blob_0 = nc.declare_dram_parameter("blob_0", [KROWS, BLOB0], BF16, isOutput=False)
    blob_z = nc.declare_dram_parameter("blob_z", [KROWS, BLOBZ], BF16, isOutput=False)
    out = nc.declare_dram_parameter("out", [N_VIEWS, 2 * NPC], F32, isOutput=True)

    with tile.TileContext(nc) as tc, ExitStack() as ctx:
        const_pool = ctx.enter_context(tc.tile_pool(name="const", bufs=1))
        psum_pool = ctx.enter_context(tc.tile_pool(name="psum", bufs=2, space="PSUM"))
        sb_pool = ctx.enter_context(tc.tile_pool(name="sb", bufs=4))
        out_pool = ctx.enter_context(tc.tile_pool(name="out", bufs=3))

        # blob layout: [weight cols ++ point cols].  Loaded in per-output-group
        # pieces so chunk 0 only waits on a few KB per row group.  Pieces are
        # issued on the ACT queue (HWDGE policy allows it) with a two-group
        # lookahead so the ~750 ns per-DMA issue cost doesn't serialize ahead
        # of the store stream on SP.
        btile = const_pool.tile([32 + KROWS, BLOB0], BF16, tag="blob")

        def piece_edges(wcols):
            edges = [0]
            acc = wcols
            for gsz in GSCHED:
                acc += gsz * CHUNK
                edges.append(acc)
            return edges

        edges0 = piece_edges(W0)
        edgesz = piece_edges(N_VIEWS)

        def load_piece(gi, split_first=False):
            if gi >= len(GSCHED):
                return
            for base, blob, e, w in (
                (0, blob_0, edges0, W0),
                (32, blob_z, edgesz, N_VIEWS),
            ):
                lo_, hi_ = e[gi], e[gi + 1]
                if split_first:
                    # weights + first chunk come in a tiny fast piece so the
                    # first matmuls start ~1.5 us earlier
                    mid = w + CHUNK
                    nc.gpsimd.dma_start(
                        btile[base : base + KROWS, lo_:mid], blob[:, lo_:mid]
                    )
                    lo_ = mid
                nc.gpsimd.dma_start(
                    btile[base : base + KROWS, lo_:hi_], blob[:, lo_:hi_]
                )

        load_piece(0, split_first=True)
        load_piece(1)

        # pre-warm the ACT spline tables (~2.7 us) under the input transfer
        warm = sb_pool.tile([1, 2], F32, tag="warm")
        nc.vector.memset(warm[:], 1.0)
        act_warm_src = warm[0:1, 0:1]
        act_warm_dst = warm[0:1, 1:2]

        ACT_FN = mybir.ActivationFunctionType

        def act_direct(out_ap, in_ap, func, bias=0.0, scale=1.0, alpha=0.0):
            # same lowering as nc.scalar.activation but without the
            # Reciprocal accuracy guard (measured 1.2e-5 rel err on our
            # [1.1, 3.6] domain, far inside the output tolerance)
            eng = nc.scalar
            ins = [eng.lower_ap(in_ap)]
            for val in (bias, scale, alpha):
                ins.append(mybir.ImmediateValue(dtype=mybir.dt.float32, value=val))
            return eng.add_instruction(
                mybir.InstActivation(
                    name=nc.get_next_instruction_name(),
                    func=func,
                    ins=ins,
                    outs=[eng.lower_ap(out_ap)],
                )
            )

        act_direct(act_warm_dst, act_warm_src, ACT_FN.Reciprocal)

        gtile = None
        gview3 = None
        g = 0            # group index
        ci = 0           # chunk index within group
        out_off = 0      # output column offset (in f32 elements)
        for c in range(CHUNKS):
            gsz = GSCHED[g]
            if ci == 0:
                load_piece(g + 2)
                gtile = out_pool.tile([N_VIEWS, 2 * gsz * CHUNK], F32, tag="g")
                # [p, two, n]: 'two' stride 1 (u,v adjacent), n stride 2
                gview3 = gtile[:].rearrange("p (n two) -> p two n", two=2)

            # U in bank 0, V in bank 1 of one PSUM tile so a single broadcast
            # tensor_tensor computes both quotients.  V starts at column 512
            # (2048 B) so each matmul output stays inside one PSUM bank.
            BANK = 512
            puv = psum_pool.tile([N_VIEWS, 2 * BANK], F32, tag="puv")
            pz = psum_pool.tile([N_VIEWS, CHUNK], F32, tag="pz")
            rhs0 = btile[0:KROWS, W0 + c * CHUNK : W0 + (c + 1) * CHUNK]
            rhsz = btile[
                32 : 32 + KROWS, N_VIEWS + c * CHUNK : N_VIEWS + (c + 1) * CHUNK
            ]
            for dst_ps, lhsT, rhs, tp in (
                (puv[:, 0:CHUNK], btile[0:KROWS, 0:N_VIEWS], rhs0, (0, 0)),
                (puv[:, BANK : BANK + CHUNK],
                 btile[0:KROWS, N_VIEWS:W0], rhs0, (0, 0)),
                (pz[:], btile[32 : 32 + KROWS, 0:N_VIEWS], rhsz, (32, 0)),
            ):
                nc.tensor.matmul(dst_ps, lhsT, rhs, tile_position=tp)

            recip = sb_pool.tile([N_VIEWS, CHUNK], F32, tag="recip")
            if clamp:
                zcl = sb_pool.tile([N_VIEWS, CHUNK], F32, tag="zcl")
                nc.vector.tensor_scalar_max(zcl[:], pz[:], -Z_MAX)
                act_direct(recip[:], zcl[:], ACT_FN.Reciprocal)
            else:
                act_direct(recip[:], pz[:], ACT_FN.Reciprocal)

            lo, hi = ci * CHUNK, (ci + 1) * CHUNK
            odst = gview3[:, :, lo:hi]                      # [p, 2, CHUNK]
            iuv = puv[:].rearrange("p (two n) -> p two n", two=2)[:, :, 0:CHUNK]
            rb = recip[:].unsqueeze(1).broadcast_to([N_VIEWS, 2, CHUNK])
            if clamp:
                tuv = sb_pool.tile([N_VIEWS, 2 * CHUNK], F32, tag="tuv")
                t3 = tuv[:].rearrange("p (two n) -> p two n", two=2)
                nc.vector.tensor_tensor(t3, iuv, rb, mybir.AluOpType.mult)
                nc.vector.tensor_scalar_add(
                    gview3[:, 0:1, lo:hi], t3[:, 0:1, :], CX
                )
                nc.vector.tensor_scalar_add(
                    gview3[:, 1:2, lo:hi], t3[:, 1:2, :], CY
                )
            else:
                nc.vector.tensor_tensor(odst, iuv, rb, mybir.AluOpType.mult)

            # store this chunk's 512 KB immediately -- keeps the DMA queues
            # fed; alternate between the two physical HWDGE rings (SP / ACT)
            dma_eng = nc.sync if c % 2 == 0 else nc.scalar
            dma_eng.dma_start(
                out[:, out_off : out_off + 2 * CHUNK],
                gtile[:, 2 * ci * CHUNK : 2 * (ci + 1) * CHUNK],
            )
            out_off += 2 * CHUNK
            ci += 1
            if ci == gsz:
                g += 1
                ci = 0

    return _install_wait_legalizer(nc)


_module_cache = {}


def _get_module(clamp):
    if clamp not in _module_cache:
        _module_cache[clamp] = _build_module(clamp)
    return _module_cache[clamp]


# ---------------------------------------------------------------------------
# Entry point
# ---------------------------------------------------------------------------
def kernel(points3d, euler_angles, translations, focal_length, _trace=False):
    points3d = np.asarray(points3d, dtype=np.float32)
    euler_angles = np.asarray(euler_angles, dtype=np.float32)
    translations = np.asarray(translations, dtype=np.float32)
    focal_length = np.asarray(focal_length, dtype=np.float32)

    # Is the Z clamp provably inactive?  znega = -(r2.p + tz) >= min_v(-tz -
    # |r2|*max|p|).  The fast path folds CX/CY into the matmul, which is only
    # exact when no point clamps.
    Rq = _euler_to_matrix(euler_angles.astype(np.float64))
    tz = translations[:, 2].astype(np.float64)
    r2n = np.linalg.norm(Rq[:, 2, :], axis=1)
    pmax = float(np.linalg.norm(points3d.astype(np.float64), axis=1).max())
    znega_lo = float((-tz - r2n * pmax).min())
    clamp = bool(znega_lo < max(-Z_MAX * 10.0, 1e-3))

    Wu, Wv, Wz = _fold_weights(euler_angles, translations, focal_length, clamp)

    import ml_dtypes

    pT = points3d.T                                   # [3, N] f32
    p_hi = pT.astype(ml_dtypes.bfloat16)              # [3, N]
    p_lo = (pT - p_hi.astype(np.float32)).astype(ml_dtypes.bfloat16)
    ones = np.ones((1, N_POINTS), dtype=ml_dtypes.bfloat16)
    pk = np.concatenate([p_hi, p_lo, p_hi, ones, ones], axis=0)  # [KROWS, N]

    nc = _get_module(clamp)
    in_maps = []
    for c in range(N_CORES):
        sl = pk[:, c * NPC : (c + 1) * NPC]
        in_maps.append(
            {
                "blob_0": np.ascontiguousarray(np.concatenate([Wu, Wv, sl], axis=1)),
                "blob_z": np.ascontiguousarray(np.concatenate([Wz, sl], axis=1)),
            }
        )

    res = run_bass_kernel_spmd(
        nc, in_maps, core_ids=list(range(N_CORES)), trace=_trace
    )

    full = np.empty((N_VIEWS, N_POINTS, 2), dtype=np.float32)
    for c in range(N_CORES):
        full[:, c * NPC : (c + 1) * NPC, :] = res.results[c]["out"].reshape(
            N_VIEWS, NPC, 2
        )
    if _trace:
        return full, res
    return full


# revision 42
# speedup vs baseline: 1.1771x; 1.1771x over previous
"""Bundle-adjustment forward projection on 8 Trainium2 NeuronCores.

reference:  R = euler_to_matrix(euler_angles)            [V,3,3]
            pc = einsum('nj,vij->vni', points3d, R) + t  [V,N,3]
            Zc = min(pc_z, -1e-4)
            u = -f*Xc/Zc + CX ; v = f*Yc/Zc + CY         -> [V,N,2]

Strategy: shard the N=200000 points across the 8 cores (25000 each); every
core computes all V=128 views so the SBUF partition dim = view is fully
utilized.  The host folds f/CX/CY/translations into three linear maps over
homogeneous points p4 = (x,y,z,1):

    U     = p4 . Wu[v]   ( =  f*Xc + CX*znega )
    Vv    = p4 . Wv[v]   ( = -f*Yc + CY*znega )
    znega = p4 . Wz[v]   ( = -Zc, always > 0 for this data )

so that u = U/znega and v = Vv/znega exactly match the reference when the
Z clamp never fires (host-verified with a rigorous bound; a clamped variant
is built instead if the bound is violated).

Per 500-point chunk: 3 PE matmuls (float32r, K=4, M=128 views, weights
resident in three PE row groups) -> PSUM; DVE: reciprocal_approx_fast on
znega, then two tensor_muls writing u,v interleaved (stride-2) straight into
the output tile; one 2.56 MB DMA per 2500 points stores [128, 5000] f32.

NOTE this walrus build accepts only ONE semaphore wait per instruction, so
the kernel is structured so every instruction has at most one unobserved
producer (points+weights share one input DMA per row group; the per-chunk
DVE stream leads with the reciprocal so PE ticks are observed before the
muls; TileContext's tail drain is patched to split its waits into nops).
"""

import numpy as np
from contextlib import ExitStack

import concourse.bass as bass
import concourse.tile as tile
from concourse import mybir
from concourse.bass_utils import run_bass_kernel_spmd
from concourse.vector_clock import ScopedClock, VectorClock

CX = 512.0
CY = 512.0
Z_MAX = -1e-4

N_CORES = 8
N_POINTS = 200000
N_VIEWS = 128
NPC = N_POINTS // N_CORES          # 25000 points per core
CHUNK = 500                        # matmul free dim (fits one PSUM bank, >=256)
CHUNKS = NPC // CHUNK              # 50
# outputs are stored per chunk (512 KB each) so the DMA queues always have
# work; gtile groups exist only for SBUF slot management
GSCHED = [5] * 10
assert sum(GSCHED) == CHUNKS
GMAX = max(GSCHED)
# blob0 carries TWO weight sections (u then v) followed by the shared point
# columns; blob_z carries one.  11-partition loads land on only 3 of the 16
# SBUF ports (~81 GB/s), so sharing one rhs replica between u and v cuts the
# input from 1.66 MB to 1.11 MB of port-bound traffic.
W0 = 2 * N_VIEWS
BLOB0 = W0 + NPC
BLOBZ = N_VIEWS + NPC
BLOB = NPC + N_VIEWS               # points ++ weight columns
# bf16 hi/lo split: K rows = [p_hi(3), p_lo(3), p_hi(3), 1, 1] against
# weight columns [w_hi(3), w_hi(3), w_lo(3), b_hi, b_lo].  All products are
# exact in the fp32 PSUM accumulate; only w_lo*p_lo (~2^-18 relative) is
# dropped -- ~30x more accurate than float32r and full PE rate.
KROWS = 11

F32 = mybir.dt.float32
BF16 = mybir.dt.bfloat16


# ---------------------------------------------------------------------------
# Tile tail-drain workaround: this walrus build only accepts ONE semaphore
# wait per CTRL instruction, but TileContext puts every outstanding proc's
# wait on the single tail Drain.  Emit one-wait nops first instead.
# ---------------------------------------------------------------------------
def _split_drain_and_barrier(self, tick_clock, wait_clock):
    gc = tick_clock.global_clock
    n = len(gc)
    for p in range(n):
        if gc[p] > 0:
            vec = [0] * n
            vec[p] = gc[p]
            nop = self.nc.sync.nop()
            wait_clock.add_sem_waits(nop.ins, ScopedClock({None: VectorClock(vec)}))
    self.nc.sync.drain()
    self.nc.all_engine_barrier()
    assert self.sems is not None
    popped = self.nc._tile_sem_poison_stack.pop()
    assert popped is self._sem_poison
    self.nc.clear_and_free_semaphores(list(self.sems.allocated().values()))
    self.nc.all_engine_barrier()


tile.TileContext._drain_and_barrier = _split_drain_and_barrier


def _legalize_waits(bir: bytes) -> bytes:
    """This walrus build accepts at most ONE semaphore wait per instruction.
    Split every multi-wait instruction by injecting same-engine NoOps (each
    carrying one wait) immediately before it: engines consume their block
    instructions in order, so the nop's wait completes before the real op."""
    import json as _json

    d = _json.loads(bir)
    ctr = 0
    for f in d["functions"]:
        for b in f["blocks"]:
            newl = []
            for inst in b["instructions"]:
                si = inst.get("sync_info")
                w = (si or {}).get("on_wait") or []
                if len(w) > 1:
                    for extra in w[:-1]:
                        ctr += 1
                        newl.append(
                            {
                                "debug": inst.get("debug", 0),
                                "engine": inst["engine"],
                                "ins": [],
                                "outs": [],
                                "name": f"I-wfix{ctr}",
                                "opcode": "NoOp",
                                "sync_info": {"on_update": [], "on_wait": [extra]},
                            }
                        )
                    si["on_wait"] = [w[-1]]
                newl.append(inst)
            b["instructions"] = newl
    return _json.dumps(d).encode()


def _install_wait_legalizer(nc):
    orig = nc.to_json_bytes

    def to_json_bytes_fixed():
        return _legalize_waits(orig())

    nc.to_json_bytes = to_json_bytes_fixed
    return nc


# ---------------------------------------------------------------------------
# Host-side math
# ---------------------------------------------------------------------------
def _euler_to_matrix(e):
    """[V,3] -> [V,3,3], Rx @ Ry @ Rz (same convention as the reference)."""
    x, y, z = e[:, 0], e[:, 1], e[:, 2]
    c1, s1 = np.cos(x), np.sin(x)
    c2, s2 = np.cos(y), np.sin(y)
    c3, s3 = np.cos(z), np.sin(z)
    zero = np.zeros_like(x)
    one = np.ones_like(x)
    Rx = np.stack([one, zero, zero, zero, c1, -s1, zero, s1, c1], -1).reshape(-1, 3, 3)
    Ry = np.stack([c2, zero, s2, zero, one, zero, -s2, zero, c2], -1).reshape(-1, 3, 3)
    Rz = np.stack([c3, -s3, zero, s3, c3, zero, zero, zero, one], -1).reshape(-1, 3, 3)
    return Rx @ Ry @ Rz


def _fold_weights(euler_angles, translations, focal_length, clamp):
    """Build the three [4, V] stationary matrices (rows x,y,z,1)."""
    R = _euler_to_matrix(euler_angles.astype(np.float64))
    t = translations.astype(np.float64)
    f = float(focal_length[0])
    r0, r1, r2 = R[:, 0, :], R[:, 1, :], R[:, 2, :]
    tx, ty, tz = t[:, 0], t[:, 1], t[:, 2]

    if clamp:
        # numerators without the CX/CY fold (added on DVE after the division)
        wU = f * r0
        bU = f * tx
        wV = -f * r1
        bV = -f * ty
    else:
        wU = f * r0 - CX * r2
        bU = f * tx - CX * tz
        wV = -f * r1 - CY * r2
        bV = -f * ty - CY * tz
    wZ = -r2
    bZ = -tz

    def pack(w, b):
        # -> [KROWS, V] bf16 lhsT: cols per view = [w_hi(3), w_hi(3), w_lo(3),
        # b_hi, b_lo] matching point rows [p_hi(3), p_lo(3), p_hi(3), 1, 1]
        import ml_dtypes

        w_hi = w.astype(ml_dtypes.bfloat16)
        w_lo = (w - w_hi.astype(np.float64)).astype(ml_dtypes.bfloat16)
        b_hi = b.astype(ml_dtypes.bfloat16)
        b_lo = (b - b_hi.astype(np.float64)).astype(ml_dtypes.bfloat16)
        return np.concatenate(
            [w_hi.T, w_hi.T, w_lo.T, b_hi[None, :], b_lo[None, :]], axis=0
        )

    return pack(wU, bU), pack(wV, bV), pack(wZ, bZ)


# ---------------------------------------------------------------------------
# Bass module
# ---------------------------------------------------------------------------
def _build_module(clamp):
    nc = bass.Bass()
    blob_0 = nc.declare_dram_parameter("blob_0", [KROWS, BLOB0], BF16, isOutput=False)
    blob_z = nc.declare_dram_parameter("blob_z", [KROWS, BLOBZ], BF16, isOutput=False)
    out = nc.declare_dram_parameter("out", [N_VIEWS, 2 * NPC], F32, isOutput=True)

    with tile.TileContext(nc) as tc, ExitStack() as ctx:
        const_pool = ctx.enter_context(tc.tile_pool(name="const", bufs=1))
        psum_pool = ctx.enter_context(tc.tile_pool(name="psum", bufs=2, space="PSUM"))
        sb_pool = ctx.enter_context(tc.tile_pool(name="sb", bufs=4))
        out_pool = ctx.enter_context(tc.tile_pool(name="out", bufs=3))

        # blob layout: [weight cols ++ point cols].  Loaded in per-output-group
        # pieces so chunk 0 only waits on a few KB per row group.  Pieces are
        # issued on the ACT queue (HWDGE policy allows it) with a two-group
        # lookahead so the ~750 ns per-DMA issue cost doesn't serialize ahead
        # of the store stream on SP.
        btile = const_pool.tile([32 + KROWS, BLOB0], BF16, tag="blob")

        def piece_edges(wcols):
            edges = [0]
            acc = wcols
            for gsz in GSCHED:
                acc += gsz * CHUNK
                edges.append(acc)
            return edges

        edges0 = piece_edges(W0)
        edgesz = piece_edges(N_VIEWS)

        def load_piece(gi, split_first=False):
            if gi >= len(GSCHED):
                return
            for base, blob, e, w in (
                (0, blob_0, edges0, W0),
                (32, blob_z, edgesz, N_VIEWS),
            ):
                lo_, hi_ = e[gi], e[gi + 1]
                if split_first:
                    # weights + first chunk come in a tiny fast piece so the
                    # first matmuls start ~1.5 us earlier
                    mid = w + CHUNK
                    nc.gpsimd.dma_start(
                        btile[base : base + KROWS, lo_:mid], blob[:, lo_:mid]
                    )
                    lo_ = mid
                nc.gpsimd.dma_start(
                    btile[base : base + KROWS, lo_:hi_], blob[:, lo_:hi_]
                )

        load_piece(0, split_first=True)
        load_piece(1)

        ACT_FN = mybir.ActivationFunctionType

        def act_direct(out_ap, in_ap, func, bias=0.0, scale=1.0, alpha=0.0):
            # same lowering as nc.scalar.activation but without the
            # Reciprocal accuracy guard (measured 1.2e-5 rel err on our
            # [1.1, 3.6] domain, far inside the output tolerance)
            eng = nc.scalar
            ins = [eng.lower_ap(in_ap)]
            for val in (bias, scale, alpha):
                ins.append(mybir.ImmediateValue(dtype=mybir.dt.float32, value=val))
            return eng.add_instruction(
                mybir.InstActivation(
                    name=nc.get_next_instruction_name(),
                    func=func,
                    ins=ins,
                    outs=[eng.lower_ap(out_ap)],
                )
            )

        gtile = None
        gview3 = None
        g = 0            # group index
        ci = 0           # chunk index within group
        out_off = 0      # output column offset (in f32 elements)
        for c in range(CHUNKS):
            gsz = GSCHED[g]
            if ci == 0:
                load_piece(g + 2)
                gtile = out_pool.tile([N_VIEWS, 2 * gsz * CHUNK], F32, tag="g")
                # [p, two, n]: 'two' stride 1 (u,v adjacent), n stride 2
                gview3 = gtile[:].rearrange("p (n two) -> p two n", two=2)

            # U in bank 0, V in bank 1 of one PSUM tile so a single broadcast
            # tensor_tensor computes both quotients.  V starts at column 512
            # (2048 B) so each matmul output stays inside one PSUM bank.
            BANK = 512
            puv = psum_pool.tile([N_VIEWS, 2 * BANK], F32, tag="puv")
            pz = psum_pool.tile([N_VIEWS, CHUNK], F32, tag="pz")
            rhs0 = btile[0:KROWS, W0 + c * CHUNK : W0 + (c + 1) * CHUNK]
            rhsz = btile[
                32 : 32 + KROWS, N_VIEWS + c * CHUNK : N_VIEWS + (c + 1) * CHUNK
            ]
            for dst_ps, lhsT, rhs, tp in (
                (puv[:, 0:CHUNK], btile[0:KROWS, 0:N_VIEWS], rhs0, (0, 0)),
                (puv[:, BANK : BANK + CHUNK],
                 btile[0:KROWS, N_VIEWS:W0], rhs0, (0, 0)),
                (pz[:], btile[32 : 32 + KROWS, 0:N_VIEWS], rhsz, (32, 0)),
            ):
                nc.tensor.matmul(dst_ps, lhsT, rhs, tile_position=tp)

            recip = sb_pool.tile([N_VIEWS, CHUNK], F32, tag="recip")
            if clamp:
                zcl = sb_pool.tile([N_VIEWS, CHUNK], F32, tag="zcl")
                nc.vector.tensor_scalar_max(zcl[:], pz[:], -Z_MAX)
                act_direct(recip[:], zcl[:], ACT_FN.Reciprocal)
            else:
                act_direct(recip[:], pz[:], ACT_FN.Reciprocal)

            lo, hi = ci * CHUNK, (ci + 1) * CHUNK
            odst = gview3[:, :, lo:hi]                      # [p, 2, CHUNK]
            iuv = puv[:].rearrange("p (two n) -> p two n", two=2)[:, :, 0:CHUNK]
            rb = recip[:].unsqueeze(1).broadcast_to([N_VIEWS, 2, CHUNK])
            if clamp:
                tuv = sb_pool.tile([N_VIEWS, 2 * CHUNK], F32, tag="tuv")
                t3 = tuv[:].rearrange("p (two n) -> p two n", two=2)
                nc.vector.tensor_tensor(t3, iuv, rb, mybir.AluOpType.mult)
                nc.vector.tensor_scalar_add(
                    gview3[:, 0:1, lo:hi], t3[:, 0:1, :], CX
                )
                nc.vector.tensor_scalar_add(
                    gview3[:, 1:2, lo:hi], t3[:, 1:2, :], CY
                )
            else:
                nc.vector.tensor_tensor(odst, iuv, rb, mybir.AluOpType.mult)

            # store this chunk's 512 KB immediately -- keeps the DMA queues fed
            nc.sync.dma_start(
                out[:, out_off : out_off + 2 * CHUNK],
                gtile[:, 2 * ci * CHUNK : 2 * (ci + 1) * CHUNK],
            )
            out_off += 2 * CHUNK
            ci += 1
            if ci == gsz:
                g += 1
                ci = 0

    return _install_wait_legalizer(nc)


_module_cache = {}


def _get_module(clamp):
    if clamp not in _module_cache:
        _module_cache[clamp] = _build_module(clamp)
    return _module_cache[clamp]


# ---------------------------------------------------------------------------
# Entry point
# ---------------------------------------------------------------------------
def kernel(points3d, euler_angles, translations, focal_length, _trace=False):
    points3d = np.asarray(points3d, dtype=np.float32)
    euler_angles = np.asarray(euler_angles, dtype=np.float32)
    translations = np.asarray(translations, dtype=np.float32)
    focal_length = np.asarray(focal_length, dtype=np.float32)

    # Is the Z clamp provably inactive?  znega = -(r2.p + tz) >= min_v(-tz -
    # |r2|*max|p|).  The fast path folds CX/CY into the matmul, which is only
    # exact when no point clamps.
    Rq = _euler_to_matrix(euler_angles.astype(np.float64))
    tz = translations[:, 2].astype(np.float64)
    r2n = np.linalg.norm(Rq[:, 2, :], axis=1)
    pmax = float(np.linalg.norm(points3d.astype(np.float64), axis=1).max())
    znega_lo = float((-tz - r2n * pmax).min())
    clamp = bool(znega_lo < max(-Z_MAX * 10.0, 1e-3))

    Wu, Wv, Wz = _fold_weights(euler_angles, translations, focal_length, clamp)

    import ml_dtypes

    pT = points3d.T                                   # [3, N] f32
    p_hi = pT.astype(ml_dtypes.bfloat16)              # [3, N]
    p_lo = (pT - p_hi.astype(np.float32)).astype(ml_dtypes.bfloat16)
    ones = np.ones((1, N_POINTS), dtype=ml_dtypes.bfloat16)
    pk = np.concatenate([p_hi, p_lo, p_hi, ones, ones], axis=0)  # [KROWS, N]

    nc = _get_module(clamp)
    in_maps = []
    for c in range(N_CORES):
        sl = pk[:, c * NPC : (c + 1) * NPC]
        in_maps.append(
            {
                "blob_0": np.ascontiguousarray(np.concatenate([Wu, Wv, sl], axis=1)),
                "blob_z": np.ascontiguousarray(np.concatenate([Wz, sl], axis=1)),
            }
        )

    res = run_bass_kernel_spmd(
        nc, in_maps, core_ids=list(range(N_CORES)), trace=_trace
    )

    full = np.empty((N_VIEWS, N_POINTS, 2), dtype=np.float32)
    for c in range(N_CORES):
        full[:, c * NPC : (c + 1) * NPC, :] = res.results[c]["out"].reshape(
            N_VIEWS, NPC, 2
        )
    if _trace:
        return full, res
    return full


# revision 44
# speedup vs baseline: 1.1866x; 1.0081x over previous
"""Bundle-adjustment forward projection on 8 Trainium2 NeuronCores.

reference:  R = euler_to_matrix(euler_angles)            [V,3,3]
            pc = einsum('nj,vij->vni', points3d, R) + t  [V,N,3]
            Zc = min(pc_z, -1e-4)
            u = -f*Xc/Zc + CX ; v = f*Yc/Zc + CY         -> [V,N,2]

Strategy: shard the N=200000 points across the 8 cores (25000 each); every
core computes all V=128 views so the SBUF partition dim = view is fully
utilized.  The host folds f/CX/CY/translations into three linear maps over
homogeneous points p4 = (x,y,z,1):

    U     = p4 . Wu[v]   ( =  f*Xc + CX*znega )
    Vv    = p4 . Wv[v]   ( = -f*Yc + CY*znega )
    znega = p4 . Wz[v]   ( = -Zc, always > 0 for this data )

so that u = U/znega and v = Vv/znega exactly match the reference when the
Z clamp never fires (host-verified with a rigorous bound; a clamped variant
is built instead if the bound is violated).

Per 500-point chunk: 3 PE matmuls (float32r, K=4, M=128 views, weights
resident in three PE row groups) -> PSUM; DVE: reciprocal_approx_fast on
znega, then two tensor_muls writing u,v interleaved (stride-2) straight into
the output tile; one 2.56 MB DMA per 2500 points stores [128, 5000] f32.

NOTE this walrus build accepts only ONE semaphore wait per instruction, so
the kernel is structured so every instruction has at most one unobserved
producer (points+weights share one input DMA per row group; the per-chunk
DVE stream leads with the reciprocal so PE ticks are observed before the
muls; TileContext's tail drain is patched to split its waits into nops).
"""

import numpy as np
from contextlib import ExitStack

import concourse.bass as bass
import concourse.tile as tile
from concourse import mybir
from concourse.bass_utils import run_bass_kernel_spmd
from concourse.vector_clock import ScopedClock, VectorClock

CX = 512.0
CY = 512.0
Z_MAX = -1e-4

N_CORES = 8
N_POINTS = 200000
N_VIEWS = 128
NPC = N_POINTS // N_CORES          # 25000 points per core
CHUNK = 500                        # matmul free dim (fits one PSUM bank, >=256)
CHUNKS = NPC // CHUNK              # 50
# outputs are stored per chunk (512 KB each) so the DMA queues always have
# work; gtile groups exist only for SBUF slot management
GSCHED = [5] * 10
assert sum(GSCHED) == CHUNKS
GMAX = max(GSCHED)
# blob0 carries TWO weight sections (u then v) followed by the shared point
# columns; blob_z carries one.  11-partition loads land on only 3 of the 16
# SBUF ports (~81 GB/s), so sharing one rhs replica between u and v cuts the
# input from 1.66 MB to 1.11 MB of port-bound traffic.
W0 = 2 * N_VIEWS
BLOB0 = W0 + NPC
BLOBZ = N_VIEWS + NPC
BLOB = NPC + N_VIEWS               # points ++ weight columns
# bf16 hi/lo split: K rows = [p_hi(3), p_lo(3), p_hi(3), 1, 1] against
# weight columns [w_hi(3), w_hi(3), w_lo(3), b_hi, b_lo].  All products are
# exact in the fp32 PSUM accumulate; only w_lo*p_lo (~2^-18 relative) is
# dropped -- ~30x more accurate than float32r and full PE rate.
KROWS = 11

F32 = mybir.dt.float32
BF16 = mybir.dt.bfloat16


# ---------------------------------------------------------------------------
# Tile tail-drain workaround: this walrus build only accepts ONE semaphore
# wait per CTRL instruction, but TileContext puts every outstanding proc's
# wait on the single tail Drain.  Emit one-wait nops first instead.
# ---------------------------------------------------------------------------
def _split_drain_and_barrier(self, tick_clock, wait_clock):
    gc = tick_clock.global_clock
    n = len(gc)
    for p in range(n):
        if gc[p] > 0:
            vec = [0] * n
            vec[p] = gc[p]
            nop = self.nc.sync.nop()
            wait_clock.add_sem_waits(nop.ins, ScopedClock({None: VectorClock(vec)}))
    self.nc.sync.drain()
    self.nc.all_engine_barrier()
    assert self.sems is not None
    popped = self.nc._tile_sem_poison_stack.pop()
    assert popped is self._sem_poison
    self.nc.clear_and_free_semaphores(list(self.sems.allocated().values()))
    self.nc.all_engine_barrier()


tile.TileContext._drain_and_barrier = _split_drain_and_barrier


def _legalize_waits(bir: bytes) -> bytes:
    """This walrus build accepts at most ONE semaphore wait per instruction.
    Split every multi-wait instruction by injecting same-engine NoOps (each
    carrying one wait) immediately before it: engines consume their block
    instructions in order, so the nop's wait completes before the real op."""
    import json as _json

    d = _json.loads(bir)
    ctr = 0
    for f in d["functions"]:
        for b in f["blocks"]:
            newl = []
            for inst in b["instructions"]:
                si = inst.get("sync_info")
                w = (si or {}).get("on_wait") or []
                if len(w) > 1:
                    for extra in w[:-1]:
                        ctr += 1
                        newl.append(
                            {
                                "debug": inst.get("debug", 0),
                                "engine": inst["engine"],
                                "ins": [],
                                "outs": [],
                                "name": f"I-wfix{ctr}",
                                "opcode": "NoOp",
                                "sync_info": {"on_update": [], "on_wait": [extra]},
                            }
                        )
                    si["on_wait"] = [w[-1]]
                newl.append(inst)
            b["instructions"] = newl
    return _json.dumps(d).encode()


def _install_wait_legalizer(nc):
    orig = nc.to_json_bytes

    def to_json_bytes_fixed():
        return _legalize_waits(orig())

    nc.to_json_bytes = to_json_bytes_fixed
    return nc


# ---------------------------------------------------------------------------
# Host-side math
# ---------------------------------------------------------------------------
def _euler_to_matrix(e):
    """[V,3] -> [V,3,3], Rx @ Ry @ Rz (same convention as the reference)."""
    x, y, z = e[:, 0], e[:, 1], e[:, 2]
    c1, s1 = np.cos(x), np.sin(x)
    c2, s2 = np.cos(y), np.sin(y)
    c3, s3 = np.cos(z), np.sin(z)
    zero = np.zeros_like(x)
    one = np.ones_like(x)
    Rx = np.stack([one, zero, zero, zero, c1, -s1, zero, s1, c1], -1).reshape(-1, 3, 3)
    Ry = np.stack([c2, zero, s2, zero, one, zero, -s2, zero, c2], -1).reshape(-1, 3, 3)
    Rz = np.stack([c3, -s3, zero, s3, c3, zero, zero, zero, one], -1).reshape(-1, 3, 3)
    return Rx @ Ry @ Rz


def _fold_weights(euler_angles, translations, focal_length, clamp):
    """Build the three [4, V] stationary matrices (rows x,y,z,1)."""
    R = _euler_to_matrix(euler_angles.astype(np.float64))
    t = translations.astype(np.float64)
    f = float(focal_length[0])
    r0, r1, r2 = R[:, 0, :], R[:, 1, :], R[:, 2, :]
    tx, ty, tz = t[:, 0], t[:, 1], t[:, 2]

    if clamp:
        # numerators without the CX/CY fold (added on DVE after the division)
        wU = f * r0
        bU = f * tx
        wV = -f * r1
        bV = -f * ty
    else:
        wU = f * r0 - CX * r2
        bU = f * tx - CX * tz
        wV = -f * r1 - CY * r2
        bV = -f * ty - CY * tz
    wZ = -r2
    bZ = -tz

    def pack(w, b):
        # -> [KROWS, V] bf16 lhsT: cols per view = [w_hi(3), w_hi(3), w_lo(3),
        # b_hi, b_lo] matching point rows [p_hi(3), p_lo(3), p_hi(3), 1, 1]
        import ml_dtypes

        w_hi = w.astype(ml_dtypes.bfloat16)
        w_lo = (w - w_hi.astype(np.float64)).astype(ml_dtypes.bfloat16)
        b_hi = b.astype(ml_dtypes.bfloat16)
        b_lo = (b - b_hi.astype(np.float64)).astype(ml_dtypes.bfloat16)
        return np.concatenate(
            [w_hi.T, w_hi.T, w_lo.T, b_hi[None, :], b_lo[None, :]], axis=0
        )

    return pack(wU, bU), pack(wV, bV), pack(wZ, bZ)


# ---------------------------------------------------------------------------
# Bass module
# ---------------------------------------------------------------------------
def _build_module(clamp):
    nc = bass.Bass()
    blob_0 = nc.declare_dram_parameter("blob_0", [KROWS, BLOB0], BF16, isOutput=False)
    blob_z = nc.declare_dram_parameter("blob_z", [KROWS, BLOBZ], BF16, isOutput=False)
    out = nc.declare_dram_parameter("out", [N_VIEWS, 2 * NPC], F32, isOutput=True)

    with tile.TileContext(nc) as tc, ExitStack() as ctx:
        const_pool = ctx.enter_context(tc.tile_pool(name="const", bufs=1))
        psum_pool = ctx.enter_context(tc.tile_pool(name="psum", bufs=2, space="PSUM"))
        sb_pool = ctx.enter_context(tc.tile_pool(name="sb", bufs=4))
        out_pool = ctx.enter_context(tc.tile_pool(name="out", bufs=3))

        # blob layout: [weight cols ++ point cols].  Loaded in per-output-group
        # pieces so chunk 0 only waits on a few KB per row group.  Pieces are
        # issued on the ACT queue (HWDGE policy allows it) with a two-group
        # lookahead so the ~750 ns per-DMA issue cost doesn't serialize ahead
        # of the store stream on SP.
        btile = const_pool.tile([32 + KROWS, BLOB0], BF16, tag="blob")

        def piece_edges(wcols):
            edges = [0]
            acc = wcols
            for gsz in GSCHED:
                acc += gsz * CHUNK
                edges.append(acc)
            return edges

        edges0 = piece_edges(W0)
        edgesz = piece_edges(N_VIEWS)

        def load_piece(gi, split_first=False):
            if gi >= len(GSCHED):
                return
            for base, blob, e, w in (
                (0, blob_0, edges0, W0),
                (32, blob_z, edgesz, N_VIEWS),
            ):
                lo_, hi_ = e[gi], e[gi + 1]
                if split_first:
                    # weights + first chunk come in a tiny fast piece so the
                    # first matmuls start ~1.5 us earlier
                    mid = w + CHUNK
                    nc.gpsimd.dma_start(
                        btile[base : base + KROWS, lo_:mid], blob[:, lo_:mid]
                    )
                    lo_ = mid
                nc.gpsimd.dma_start(
                    btile[base : base + KROWS, lo_:hi_], blob[:, lo_:hi_]
                )

        load_piece(0, split_first=True)
        load_piece(1)

        ACT_FN = mybir.ActivationFunctionType

        def act_direct(out_ap, in_ap, func, bias=0.0, scale=1.0, alpha=0.0):
            # same lowering as nc.scalar.activation but without the
            # Reciprocal accuracy guard (measured 1.2e-5 rel err on our
            # [1.1, 3.6] domain, far inside the output tolerance)
            eng = nc.scalar
            ins = [eng.lower_ap(in_ap)]
            for val in (bias, scale, alpha):
                ins.append(mybir.ImmediateValue(dtype=mybir.dt.float32, value=val))
            return eng.add_instruction(
                mybir.InstActivation(
                    name=nc.get_next_instruction_name(),
                    func=func,
                    ins=ins,
                    outs=[eng.lower_ap(out_ap)],
                )
            )

        # pre-warm the ACT spline tables (~2.7 us) under the input transfer:
        # the PSEUDO_LOAD_ACT_FUNC_SET is inserted before the first ACTIVATE,
        # so issue a 1-element Reciprocal before the pipeline needs one
        warm = sb_pool.tile([1, 2], F32, tag="warm")
        nc.vector.memset(warm[:], 1.0)
        act_direct(warm[0:1, 1:2], warm[0:1, 0:1], ACT_FN.Reciprocal)

        gtile = None
        gview3 = None
        g = 0            # group index
        ci = 0           # chunk index within group
        out_off = 0      # output column offset (in f32 elements)
        for c in range(CHUNKS):
            gsz = GSCHED[g]
            if ci == 0:
                load_piece(g + 2)
                gtile = out_pool.tile([N_VIEWS, 2 * gsz * CHUNK], F32, tag="g")
                # [p, two, n]: 'two' stride 1 (u,v adjacent), n stride 2
                gview3 = gtile[:].rearrange("p (n two) -> p two n", two=2)

            # U in bank 0, V in bank 1 of one PSUM tile so a single broadcast
            # tensor_tensor computes both quotients.  V starts at column 512
            # (2048 B) so each matmul output stays inside one PSUM bank.
            BANK = 512
            puv = psum_pool.tile([N_VIEWS, 2 * BANK], F32, tag="puv")
            pz = psum_pool.tile([N_VIEWS, CHUNK], F32, tag="pz")
            rhs0 = btile[0:KROWS, W0 + c * CHUNK : W0 + (c + 1) * CHUNK]
            rhsz = btile[
                32 : 32 + KROWS, N_VIEWS + c * CHUNK : N_VIEWS + (c + 1) * CHUNK
            ]
            for dst_ps, lhsT, rhs, tp in (
                (puv[:, 0:CHUNK], btile[0:KROWS, 0:N_VIEWS], rhs0, (0, 0)),
                (puv[:, BANK : BANK + CHUNK],
                 btile[0:KROWS, N_VIEWS:W0], rhs0, (0, 0)),
                (pz[:], btile[32 : 32 + KROWS, 0:N_VIEWS], rhsz, (32, 0)),
            ):
                nc.tensor.matmul(dst_ps, lhsT, rhs, tile_position=tp)

            recip = sb_pool.tile([N_VIEWS, CHUNK], F32, tag="recip")
            if clamp:
                zcl = sb_pool.tile([N_VIEWS, CHUNK], F32, tag="zcl")
                nc.vector.tensor_scalar_max(zcl[:], pz[:], -Z_MAX)
                act_direct(recip[:], zcl[:], ACT_FN.Reciprocal)
            else:
                act_direct(recip[:], pz[:], ACT_FN.Reciprocal)

            lo, hi = ci * CHUNK, (ci + 1) * CHUNK
            odst = gview3[:, :, lo:hi]                      # [p, 2, CHUNK]
            iuv = puv[:].rearrange("p (two n) -> p two n", two=2)[:, :, 0:CHUNK]
            rb = recip[:].unsqueeze(1).broadcast_to([N_VIEWS, 2, CHUNK])
            if clamp:
                tuv = sb_pool.tile([N_VIEWS, 2 * CHUNK], F32, tag="tuv")
                t3 = tuv[:].rearrange("p (two n) -> p two n", two=2)
                nc.vector.tensor_tensor(t3, iuv, rb, mybir.AluOpType.mult)
                nc.vector.tensor_scalar_add(
                    gview3[:, 0:1, lo:hi], t3[:, 0:1, :], CX
                )
                nc.vector.tensor_scalar_add(
                    gview3[:, 1:2, lo:hi], t3[:, 1:2, :], CY
                )
            else:
                nc.vector.tensor_tensor(odst, iuv, rb, mybir.AluOpType.mult)

            # store this chunk's 512 KB immediately -- keeps the DMA queues fed
            nc.sync.dma_start(
                out[:, out_off : out_off + 2 * CHUNK],
                gtile[:, 2 * ci * CHUNK : 2 * (ci + 1) * CHUNK],
            )
            out_off += 2 * CHUNK
            ci += 1
            if ci == gsz:
                g += 1
                ci = 0

    return _install_wait_legalizer(nc)


_module_cache = {}


def _get_module(clamp):
    if clamp not in _module_cache:
        _module_cache[clamp] = _build_module(clamp)
    return _module_cache[clamp]


# ---------------------------------------------------------------------------
# Entry point
# ---------------------------------------------------------------------------
def kernel(points3d, euler_angles, translations, focal_length, _trace=False):
    points3d = np.asarray(points3d, dtype=np.float32)
    euler_angles = np.asarray(euler_angles, dtype=np.float32)
    translations = np.asarray(translations, dtype=np.float32)
    focal_length = np.asarray(focal_length, dtype=np.float32)

    # Is the Z clamp provably inactive?  znega = -(r2.p + tz) >= min_v(-tz -
    # |r2|*max|p|).  The fast path folds CX/CY into the matmul, which is only
    # exact when no point clamps.
    Rq = _euler_to_matrix(euler_angles.astype(np.float64))
    tz = translations[:, 2].astype(np.float64)
    r2n = np.linalg.norm(Rq[:, 2, :], axis=1)
    pmax = float(np.linalg.norm(points3d.astype(np.float64), axis=1).max())
    znega_lo = float((-tz - r2n * pmax).min())
    clamp = bool(znega_lo < max(-Z_MAX * 10.0, 1e-3))

    Wu, Wv, Wz = _fold_weights(euler_angles, translations, focal_length, clamp)

    import ml_dtypes

    pT = points3d.T                                   # [3, N] f32
    p_hi = pT.astype(ml_dtypes.bfloat16)              # [3, N]
    p_lo = (pT - p_hi.astype(np.float32)).astype(ml_dtypes.bfloat16)
    ones = np.ones((1, N_POINTS), dtype=ml_dtypes.bfloat16)
    pk = np.concatenate([p_hi, p_lo, p_hi, ones, ones], axis=0)  # [KROWS, N]

    nc = _get_module(clamp)
    in_maps = []
    for c in range(N_CORES):
        sl = pk[:, c * NPC : (c + 1) * NPC]
        in_maps.append(
            {
                "blob_0": np.ascontiguousarray(np.concatenate([Wu, Wv, sl], axis=1)),
                "blob_z": np.ascontiguousarray(np.concatenate([Wz, sl], axis=1)),
            }
        )

    res = run_bass_kernel_spmd(
        nc, in_maps, core_ids=list(range(N_CORES)), trace=_trace
    )

    full = np.empty((N_VIEWS, N_POINTS, 2), dtype=np.float32)
    for c in range(N_CORES):
        full[:, c * NPC : (c + 1) * NPC, :] = res.results[c]["out"].reshape(
            N_VIEWS, NPC, 2
        )
    if _trace:
        return full, res
    return full
